# revision 41
# baseline (speedup 1.0000x reference)
"""Self-contained Trainium2 Bass kernel for BertSelfAttention (relative_key_query).

kernel(**inputs) takes FULL unsharded inputs (as in setup_inputs()) and returns
the FULL (8, 1024, 1024) float32 output. Internally: data-parallel over the
batch dimension, one batch per NeuronCore across 8 cores, via
concourse run_bass_kernel_spmd.

v2: merged DMAs (window writes / skewed reads / weight loads), weights
preloaded and relaid out on host, bf16 activations/probs, double-buffered
score PSUM, balanced PSUM->SBUF evacuation across ACT/DVE.
"""

import os
import numpy as np


import concourse.bacc as bacc
import concourse.mybir as mybir
import concourse.tile as tile

f32 = mybir.dt.float32
f32r = mybir.dt.float32r
bf16 = mybir.dt.bfloat16
fp8 = mybir.dt.float8e4

S = 1024
D = 1024
H = 16
DH = 64
NT = 8
WIN = 1152
NPAIR = 8


def host_prep(hidden_states, attention_mask, W_qkv, b_qkv, dist_emb):
    import ml_dtypes

    B = hidden_states.shape[0]
    W = np.asarray(W_qkv, dtype=np.float32)
    b = np.asarray(b_qkv, dtype=np.float32)
    T = np.asarray(dist_emb, dtype=np.float32)

    # qk column selection: partition j in 0..127 <-> (hh = j>=64, d = j%64)
    # for head h = 2*t + hh; chunk ct = t for q, 8+t for k.
    qcols = np.zeros((8, 128), dtype=np.int64)
    kcols = np.zeros((8, 128), dtype=np.int64)
    for t in range(8):
        for j in range(128):
            h = 2 * t + (j >= 64)
            d = j % 64
            qcols[t, j] = h * 192 + d
            kcols[t, j] = h * 192 + 64 + d
    qk_idx = np.concatenate([qcols.reshape(-1), kcols.reshape(-1)])
    WQK = np.ascontiguousarray(W[:, qk_idx])  # [1024, 2048]
    bQK = np.ascontiguousarray(b[qk_idx].reshape(16, 128).T)  # [128, 16]

    # wqk relayout: per pair P one contiguous [128, 2048] block:
    # wqkr[p, ((dst*8 + it)*128 + c)] = WQK[it*128 + p, 128*(8*dst + P) + c]
    WQKr = np.zeros((128, 16 * 2048), dtype=np.float32)
    for P in range(8):
        for dst in range(2):
            for it in range(8):
                blk = WQK[128 * it:128 * (it + 1),
                          128 * (8 * dst + P):128 * (8 * dst + P) + 128]
                WQKr[:, 2048 * P + (dst * 8 + it) * 128:
                     2048 * P + (dst * 8 + it) * 128 + 128] = blk
    WQKr = WQKr.astype(ml_dtypes.bfloat16)

    vidx = np.array([h * 192 + 128 + d for h in range(H) for d in range(DH)])
    WV = np.ascontiguousarray(W[:, vidx])  # [1024, 1024]
    bV = np.ascontiguousarray(b[vidx].reshape(1, 1024))
    # wv relayout bf16: wvr[p, 1024*it + j] = WV[128*it + p, j]
    WVr = np.zeros((128, 8192), dtype=np.float32)
    for it in range(8):
        WVr[:, 1024 * it:1024 * (it + 1)] = WV[128 * it:128 * (it + 1), :]
    WVr = WVr.astype(ml_dtypes.bfloat16)

    T2 = np.zeros((128, 2048), dtype=np.float32)
    T2[0:64, 0:2047] = T.T
    T2[64:128, 0:2047] = T.T
    T2R = np.zeros((128, 2048), dtype=np.float32)
    T2R[0:64, 0:2047] = T.T[:, ::-1]
    T2R[64:128, 0:2047] = T.T[:, ::-1]

    ones_r = np.ones((1, 128), dtype=np.float32)
    id8_h = np.eye(128, dtype=np.float32).astype(ml_dtypes.float8_e4m3fn)
    idf_h = np.eye(128, dtype=np.float32).astype(ml_dtypes.bfloat16)

    mask = np.asarray(attention_mask, dtype=np.float32).reshape(B, S)
    in_maps = []
    for bi in range(B):
        mhat = np.ascontiguousarray(mask[bi].reshape(8, 128).T)
        in_maps.append({
            "hs": np.ascontiguousarray(hidden_states[bi],
                                       dtype=np.float32).astype(
                                           ml_dtypes.bfloat16),
            "wqk": WQKr, "bqk": bQK, "wv": WVr, "bv": bV,
            "t2": T2, "t2r": T2R, "ones_r": ones_r, "mhat": mhat,
            "id8_h": id8_h, "idf_h": idf_h,
        })
    return in_maps


def build_program(npair=NPAIR):
    nc = bacc.Bacc()
    hs_d = nc.declare_dram_parameter("hs", [S, D], bf16, isOutput=False)
    wqk_d = nc.declare_dram_parameter("wqk", [128, 16 * 2048], bf16, isOutput=False)
    bqk_d = nc.declare_dram_parameter("bqk", [128, 16], f32, isOutput=False)
    wv_d = nc.declare_dram_parameter("wv", [128, 8192], bf16, isOutput=False)
    bv_d = nc.declare_dram_parameter("bv", [1, 1024], f32r, isOutput=False)
    t2_d = nc.declare_dram_parameter("t2", [128, 2048], f32r, isOutput=False)
    t2r_d = nc.declare_dram_parameter("t2r", [128, 2048], f32r, isOutput=False)
    ones_d = nc.declare_dram_parameter("ones_r", [1, 128], f32r, isOutput=False)
    mhat_d = nc.declare_dram_parameter("mhat", [128, 8], f32, isOutput=False)
    id8_d = nc.declare_dram_parameter("id8_h", [128, 128], fp8, isOutput=False)
    idf_d = nc.declare_dram_parameter("idf_h", [128, 128], bf16, isOutput=False)
    out_d = nc.declare_dram_parameter("out", [S, D], f32, isOutput=True)

    Exp = mybir.ActivationFunctionType.Exp
    Ident = mybir.ActivationFunctionType.Identity

    with tile.TileContext(nc) as tc:
        with tc.tile_pool(name="const", bufs=1) as cpool:
            t2_sb = cpool.tile([128, 2048], f32r, tag="t2", name="t2")
            t2r_sb = cpool.tile([128, 2048], f32r, tag="t2r", name="t2r")
            bqk_sb = cpool.tile([128, 16], f32, tag="bqk", name="bqk")
            bv_sb = cpool.tile([1, 1024], f32r, tag="bv", name="bv")
            ones_sb = cpool.tile([1, 128], f32r, tag="ones", name="ones")
            mhat_sb = cpool.tile([128, 8], f32, tag="mh", name="mh")
            id8 = cpool.tile([128, 128], fp8, tag="id8", name="id8")
            idf = cpool.tile([128, 128], bf16, tag="idf", name="idf")
            ones16 = cpool.tile([128, 16], bf16, tag="o16", name="o16")
            nc.vector.memset(ones16[:], 1.0)

            # resident across phase C
            wqk_tiles = [cpool.tile([128, 2048], bf16, tag=f"wqk{i}",
                                    name=f"wqk{i}") for i in range(2)]
            hsT = cpool.tile([128, 8192], bf16, tag="hsT", name="hsT")
            vh = [cpool.tile([128, 1040], bf16, tag=f"vh{t}", name=f"vh{t}")
                  for t in range(NT)]
            wv_sb = cpool.tile([128, 8192], bf16, tag="wv", name="wv")

            # ---- Phase A: hs -> hsT (bf16) via PE transpose ----
            with tc.tile_pool(name="pA", bufs=1) as pA, \
                 tc.tile_pool(name="pAps", bufs=8, space="PSUM") as pAps:
                hs_all = pA.tile([128, 8192], bf16, tag="hsall", name="hsall")
                # per-tile DMAs: hs_all[p, 1024*t + c] = hs[128*t + p, c]
                nc.sync.dma_start(
                    hs_all[:, 0:1024], hs_d.ap()[0:128, :])
                nc.sync.dma_start(idf[:], idf_d.ap())
                for t_ in range(1, NT):
                    nc.sync.dma_start(
                        hs_all[:, 1024 * t_:1024 * (t_ + 1)],
                        hs_d.ap()[128 * t_:128 * (t_ + 1), :])
                nc.sync.dma_start(wqk_tiles[0][:], wqk_d.ap()[:, 0:2048])
                nc.sync.dma_start(bqk_sb[:], bqk_d.ap())
                for it_ in range(NT):
                    nc.sync.dma_start(
                        wv_sb[:, 1024 * it_:1024 * (it_ + 1)],
                        wv_d.ap()[:, 1024 * it_:1024 * (it_ + 1)])
                nc.sync.dma_start(bv_sb[:], bv_d.ap())
                nc.sync.dma_start(ones_sb[:], ones_d.ap())
                nc.sync.dma_start(t2_sb[:], t2_d.ap())
                nc.sync.dma_start(t2r_sb[:], t2r_d.ap())
                nc.sync.dma_start(id8[:], id8_d.ap())
                nc.sync.dma_start(mhat_sb[:], mhat_d.ap())
                for ig in range(2):
                    for lt in range(NT):
                        ps = pAps.tile([128, 512], bf16, tag="tp", name="tp")
                        for j in range(4):
                            it = 4 * ig + j
                            nc.tensor.matmul(
                                ps[:, 128 * j:128 * (j + 1)],
                                hs_all[:, 1024 * lt + 128 * it:
                                       1024 * lt + 128 * (it + 1)],
                                idf[:],
                                is_transpose=True,
                                start=(j == 0), stop=(j == 3),
                                skip_group_check=True)
                        # hsT[p, 1024*it + 128*lt + q] = ps[p, 128*j + q]
                        out_ap = hsT[:].__replace__(
                            ap=[[8192, 128], [1024, 4], [1, 128]],
                            offset=1024 * (4 * ig) + 128 * lt)
                        in_ap = ps[:].__replace__(
                            ap=[[512, 128], [128, 4], [1, 128]], offset=0)
                        if lt % 2 == 0:
                            nc.scalar.copy(out_ap, in_ap)
                        else:
                            nc.vector.tensor_copy(out_ap, in_ap)

            # ---- Phase C: per head-pair, fine-grained interleaved pipeline ----
            with tc.tile_pool(name="qk", bufs=2) as qkpool, \
                 tc.tile_pool(name="w8p", bufs=1) as w8p, \
                 tc.tile_pool(name="s12", bufs=1) as s12pool, \
                 tc.tile_pool(name="probs", bufs=16) as prpool, \
                 tc.tile_pool(name="osmall", bufs=8) as osmall, \
                 tc.tile_pool(name="oap", bufs=2) as oapool, \
                 tc.tile_pool(name="dram", bufs=3, space="DRAM") as dpool, \
                 tc.tile_pool(name="bigps", bufs=3, space="PSUM") as bigps, \
                 tc.tile_pool(name="ctxps", bufs=1, space="PSUM") as ctxps, \
                 tc.tile_pool(name="winps", bufs=3, space="PSUM") as winps:

                def make_b_units():
                    units = []

                    def b_unit(tau):
                        def f():
                            for half in range(2):
                                sl = slice(512 * half, 512 * (half + 1))
                                psv = bigps.tile([128, 512], f32, tag="big",
                                                 name="vps")
                                nc.tensor.matmul(psv[:, 0:512], ones_sb[:],
                                                 bv_sb[:, sl],
                                                 start=True, stop=False,
                                                 skip_group_check=True)
                                for it in range(NT):
                                    nc.tensor.matmul(
                                        psv[:, 0:512],
                                        hsT[:, 1024 * it + 128 * tau:
                                            1024 * it + 128 * tau + 128],
                                        wv_sb[:, 1024 * it + 512 * half:
                                              1024 * it + 512 * (half + 1)],
                                        start=False, stop=(it == NT - 1),
                                        skip_group_check=True)
                                out_ap = vh[tau][:].__replace__(
                                    ap=[[1040, 128], [65, 8], [1, 64]],
                                    offset=65 * 8 * half)
                                in_ap = psv[:].__replace__(
                                    ap=[[512, 128], [64, 8], [1, 64]],
                                    offset=0)
                                if (tau + half) % 2 == 0:
                                    nc.scalar.copy(out_ap, in_ap)
                                else:
                                    nc.vector.tensor_copy(out_ap, in_ap)
                            ones_ap = vh[tau][:].__replace__(
                                ap=[[1040, 128], [65, 16]], offset=64)
                            nc.scalar.copy(ones_ap, ones16[:])
                        return f

                    for tau in range(NT):
                        units.append(b_unit(tau))
                    return units

                def dma_wqk(P):
                    wqk_sb = wqk_tiles[P % 2]
                    nc.sync.dma_start(wqk_sb[:],
                                      wqk_d.ap()[:, 2048 * P:2048 * (P + 1)])
                    return wqk_sb

                def emit_3a(P, wqk_sb):
                    qT = qkpool.tile([128, 1024], f32r, tag="qT", name="qT")
                    kT = qkpool.tile([128, 1024], f32r, tag="kT", name="kT")
                    for di, dst in enumerate((qT, kT)):
                        ct = 8 * di + P
                        for half in range(2):
                            sl = slice(512 * half, 512 * (half + 1))
                            ps = bigps.tile([128, 512], f32, tag="big",
                                            name="qkps")
                            for it in range(NT):
                                nc.tensor.matmul(
                                    ps[:, 0:512],
                                    wqk_sb[:, (di * 8 + it) * 128:
                                           (di * 8 + it) * 128 + 128],
                                    hsT[:, 1024 * it + 512 * half:
                                        1024 * it + 512 * (half + 1)],
                                    start=(it == 0), stop=(it == NT - 1),
                                    skip_group_check=True)
                            nc.scalar.activation(dst[:, sl], ps[:, 0:512],
                                                 Ident,
                                                 bias=bqk_sb[:, ct:ct + 1],
                                                 scale=1.0)
                    return qT, kT

                def make_ab_units(P, qT, kT):
                    """3b window units: list of thunks; each computes one
                    (hh, qk, t) window tile (3 MMs + 3 copies + 1 DMA out);
                    skew-read prefetch thunks appended per hh."""
                    units = []
                    s1t = [None, None]
                    s2t = [None, None]
                    w8s = {}
                    dqs = {}
                    for hh in range(2):
                        for qk in range(2):
                            w8s[(hh, qk)] = w8p.tile(
                                [128, 8 * WIN], fp8,
                                tag=f"w8_{hh}{qk}", name=f"w8_{hh}{qk}")
                            dqs[(hh, qk)] = dpool.tile(
                                [128, 8 * WIN], fp8,
                                tag=f"d{hh}{qk}", name=f"d{hh}{qk}")

                    def win_unit(hh, qk, t, c3s):
                        def f():
                            rs = slice(64 * hh, 64 * (hh + 1))
                            base = 896 - 128 * t
                            src_sb = qT if qk == 0 else kT
                            tbl = t2r_sb if qk == 0 else t2_sb
                            w8 = w8s[(hh, qk)]
                            for c3 in c3s:
                                wps = winps.tile([128, 512], f32,
                                                 tag="winps", name="winps")
                                nc.tensor.matmul(
                                    wps[:, 0:384],
                                    src_sb[rs, 128 * t:128 * (t + 1)],
                                    tbl[rs, base + 384 * c3:
                                        base + 384 * (c3 + 1)],
                                    start=True, stop=True,
                                    skip_group_check=True)
                                on_dve = (qk, c3) in ((0, 0), (0, 1), (1, 2)) \
                                    or ((qk, c3) == (0, 2) and t % 2 == 1)
                                if on_dve:
                                    nc.vector.tensor_copy(
                                        w8[:, WIN * t + 384 * c3:
                                           WIN * t + 384 * (c3 + 1)],
                                        wps[:, 0:384])
                                else:
                                    nc.scalar.copy(
                                        w8[:, WIN * t + 384 * c3:
                                           WIN * t + 384 * (c3 + 1)],
                                        wps[:, 0:384])
                            if 2 in c3s:
                                nc.gpsimd.dma_start(
                                    dqs[(hh, qk)][:, WIN * t:WIN * (t + 1)],
                                    w8[:, WIN * t:WIN * (t + 1)])
                        return f

                    def skew_read(hh):
                        def f():
                            s1 = s12pool.tile([128, 8192], fp8,
                                              tag=f"s1h{hh}", name=f"s1h{hh}")
                            nc.sync.dma_start(
                                s1[:], dqs[(hh, 0)][:].__replace__(
                                    ap=[[9215, 128], [WIN, 8], [1, 1024]],
                                    offset=127))
                            s2 = s12pool.tile([128, 8192], fp8,
                                              tag=f"s2h{hh}", name=f"s2h{hh}")
                            nc.sync.dma_start(
                                s2[:], dqs[(hh, 1)][:].__replace__(
                                    ap=[[9215, 128], [WIN, 8], [1, 1024]],
                                    offset=127))
                            s1t[hh] = s1
                            s2t[hh] = s2
                        return f

                    for hh in range(2):
                        for t in range(NT):
                            units.append(win_unit(hh, 0, t, (0, 1)))
                            units.append(win_unit(hh, 0, t, (2,)))
                            units.append(win_unit(hh, 1, t, (0, 1)))
                            units.append(win_unit(hh, 1, t, (2,)))
                        units.append(skew_read(hh))
                    return units, (P, s1t, s2t)

                def make_c_units(state, qT, kT):
                    """score/softmax/ctx units for pair P (uses its own qT/kT)."""
                    P, s1t, s2t = state
                    units = []
                    oat = [oapool.tile([128, 128], f32, tag=f"oa{L}",
                                       name=f"oa{L}") for L in range(NT)]
                    for hh in range(2):
                        h = 2 * P + hh
                        rs = slice(64 * hh, 64 * (hh + 1))
                        prs = {}
                        pcs = {}

                        def sc_unit(hh, t, half, prs=prs):
                            def f():
                                rs = slice(64 * hh, 64 * (hh + 1))
                                s1 = s1t[hh]
                                s2 = s2t[hh]
                                sc = bigps.tile([128, 512], f32, tag="big",
                                                name="sc")
                                for Lh in range(4):
                                    L = 4 * half + Lh
                                    nc.tensor.matmul(
                                        sc[:, 128 * Lh:128 * (Lh + 1)],
                                        s1[:, 1024 * L + 128 * t:
                                           1024 * L + 128 * (t + 1)],
                                        id8[:],
                                        start=(Lh == 0), stop=False,
                                        skip_group_check=True)
                                nc.tensor.matmul(
                                    sc[:, 0:512],
                                    kT[rs, 128 * t:128 * (t + 1)],
                                    qT[rs, 512 * half:512 * (half + 1)],
                                    start=False, stop=False,
                                    skip_group_check=True)
                                nc.tensor.matmul(
                                    sc[:, 0:512], id8[:],
                                    s2[:, 1024 * t + 512 * half:
                                       1024 * t + 512 * (half + 1)],
                                    start=False, stop=True,
                                    skip_group_check=True)
                                pr = prpool.tile([128, 512], bf16, tag="pr",
                                                 name="pr")
                                nc.scalar.activation(pr[:], sc[:, 0:512], Exp,
                                                     bias=mhat_sb[:, t:t + 1],
                                                     scale=0.125)
                                prs[(t, half)] = pr
                            return f

                        def ctx_asm(hh, h, Lg, first, P=P, prs=prs,
                                    pcs=pcs):
                            def f():
                                if first:
                                    pcs[0] = ctxps.tile([128, 1024], f32,
                                                        tag="ctx", name="ctx")
                                pc = pcs[0]
                                for L in range(Lg, Lg + 2):
                                    for t in range(NT):
                                        nc.tensor.matmul(
                                            pc[:, 128 * L:128 * L + 65],
                                            prs[(t, L // 4)][:,
                                                128 * (L % 4):
                                                128 * (L % 4) + 128],
                                            vh[t][:, 65 * h:65 * (h + 1)],
                                            start=(t == 0), stop=(t == NT - 1),
                                            skip_group_check=True)
                                for L in range(Lg - 4, Lg - 2):
                                    if L < 0:
                                        continue
                                    rec = osmall.tile([128, 1], f32, tag="rec",
                                                      name="rec")
                                    nc.vector.reciprocal(
                                        rec[:],
                                        pc[:, 128 * L + 64:128 * L + 65])
                                    nc.vector.tensor_scalar_mul(
                                        oat[L][:, 64 * hh:64 * (hh + 1)],
                                        pc[:, 128 * L:128 * L + 64], rec[:])
                                    if hh == 1:
                                        dst = out_d.ap().__replace__(
                                            ap=[[1024, 128], [1, 128]],
                                            offset=128 * L * 1024 + 128 * P)
                                        nc.sync.dma_start(dst, oat[L][:])
                            return f

                        for t in range(NT):
                            for half in range(2):
                                units.append(sc_unit(hh, t, half))
                        def asm_tail(hh, h, P=P, pcs=pcs):
                            def f():
                                pc = pcs[0]
                                for L in range(4, NT):
                                    rec = osmall.tile([128, 1], f32, tag="rec",
                                                      name="rec")
                                    nc.vector.reciprocal(
                                        rec[:],
                                        pc[:, 128 * L + 64:128 * L + 65])
                                    nc.vector.tensor_scalar_mul(
                                        oat[L][:, 64 * hh:64 * (hh + 1)],
                                        pc[:, 128 * L:128 * L + 64], rec[:])
                                    if hh == 1:
                                        dst = out_d.ap().__replace__(
                                            ap=[[1024, 128], [1, 128]],
                                            offset=128 * L * 1024 + 128 * P)
                                        nc.sync.dma_start(dst, oat[L][:])
                            return f

                        for Lg in (0, 2, 4, 6):
                            units.append(ctx_asm(hh, h, Lg, Lg == 0))
                        units.append(asm_tail(hh, h))
                    return units

                wqk_sb = wqk_tiles[0]
                prev_c_units = make_b_units()
                for P in range(npair):
                    qT, kT = emit_3a(P, wqk_sb)
                    if P + 1 < npair:
                        wqk_sb = dma_wqk(P + 1)
                    ab_units, state = make_ab_units(P, qT, kT)
                    # interleave this pair's window units with previous pair's
                    # score units (pair 0: with the v-hat units)
                    ai, ci = 0, 0
                    while ai < len(ab_units) or ci < len(prev_c_units):
                        if ci < len(prev_c_units):
                            prev_c_units[ci]()
                            ci += 1
                        for _ in range(200):
                            if ai < len(ab_units):
                                ab_units[ai]()
                                ai += 1
                    prev_c_units = make_c_units(state, qT, kT)
                for u in prev_c_units:
                    u()

    nc.compile()
    return nc


def run_cores(nc, in_maps, core_ids=None, trace=False):
    from concourse.bass_utils import run_bass_kernel_spmd
    if core_ids is None:
        core_ids = list(range(len(in_maps)))
    return run_bass_kernel_spmd(nc, in_maps, core_ids, trace=trace)


_NC_CACHE = {}
_LAST = {"exec_time_ns": None}


def _get_program():
    if "nc" not in _NC_CACHE:
        _NC_CACHE["nc"] = build_program()
    return _NC_CACHE["nc"]


def get_last_exec_time_ns():
    return _LAST["exec_time_ns"]


def kernel(hidden_states, attention_mask, W_qkv, b_qkv, dist_emb):
    from concourse.bass_utils import run_bass_kernel_spmd

    hidden_states = np.asarray(hidden_states, dtype=np.float32)
    attention_mask = np.asarray(attention_mask, dtype=np.float32)
    W_qkv = np.asarray(W_qkv, dtype=np.float32)
    b_qkv = np.asarray(b_qkv, dtype=np.float32)
    dist_emb = np.asarray(dist_emb, dtype=np.float32)

    B = hidden_states.shape[0]
    nc = _get_program()
    in_maps = host_prep(hidden_states, attention_mask, W_qkv, b_qkv, dist_emb)
    trace = bool(os.environ.get("BASS_TRACE"))
    res = run_bass_kernel_spmd(nc, in_maps, list(range(B)), trace=trace)
    _LAST["exec_time_ns"] = res.exec_time_ns
    out = np.stack([res.results[i]["out"] for i in range(B)], axis=0)
    return out.astype(np.float32)
